# revision 3
# baseline (speedup 1.0000x reference)
"""Trainium2 Bass kernel for nn_Criterion_37984690765901.

Loss =  L_t + lam_e * Loss_e + lam_od * (L_zt + L_zs)
  L_t    = mean_r( lse(y_zt_r) - y_zt[r, target_r] )            (cross entropy)
  Loss_e = mean_r( lse(s_r) - (sum_j e^{s_rj} s_rj)/sum_j e^{s_rj} )   (entropy)
  L_zt/L_zs = mean_r( rowdot_r/s_r - ln s_r + ln ps_r )          (KLD batchmean)
     with enc = mean + exp(0.5*log_std)*eps,  e = exp(enc), s = sum_d e,
     pe = exp(prior), ps = sum_d pe, rowdot = sum_d e*(enc - prior).
     (prior_s = 1 + eps_prior_s, but KLD is shift-invariant in the prior
      logits, so eps_prior_s is used directly.)

Sharding: pure data parallel over the batch axis, 8192 rows per core.
Each [8192, D] shard is viewed as [128, 8192] (partition p holds rows
64p..64p+63 contiguously); all per-row reductions are free-axis segmented
reduces, and the batch reduction is finished on the host in float64.

Scheduling note: walrus allows a single sync-wait command per DVE
instruction, so the per-chunk op order is arranged such that every
instruction needs at most one unobserved cross-engine semaphore (the
PS-reduce observes ACT first; y_zt and its one-hot ride one DMA).

Device per-core outputs: out[128, 256] f32 =
  [:, 0:64]    per-row KL contribution, t branch
  [:, 64:128]  per-row KL contribution, s branch
  [:, 128:192] per-row (lse_y - y_pick)
  [:, 192:256] per-row entropy of softmax(s_zt)
"""

import os
import numpy as np

NCORES = 8
B, D, C, S = 65536, 128, 10, 2
LAMBDA_E, LAMBDA_OD = 0.1, 0.036
GAMMA_E, GAMMA_OD = 2.0, 2.0
STEP_SIZE = 1000.0

RPC = B // NCORES            # rows per core = 8192
P = 128                      # SBUF partitions
FREE = RPC * D // P          # 8192 free elems per partition per big tensor
CHUNK = 2048                 # free elems per chunk
G = CHUNK // D               # 16 row-groups per chunk
NCH = FREE // CHUNK          # 4 chunks per tensor
NCOL = FREE // D             # 64 rows per partition (stat columns)
YF = RPC * C // P            # 640
SF = RPC * S // P            # 128

# (mean, log_std, eps, prior) DRAM names per branch
BRANCHES = [
    ("mt", "lt", "et", "ept"),
    ("ms", "lsd", "ess", "eps2"),
]

_CACHED_NC = None
LAST_EXEC_NS = None


def _build_nc():
    import concourse.bass as bass
    import concourse.tile as tile
    from concourse import mybir
    from contextlib import ExitStack

    f32 = mybir.dt.float32
    Exp = mybir.ActivationFunctionType.Exp
    Ln = mybir.ActivationFunctionType.Ln
    add = mybir.AluOpType.add
    sub = mybir.AluOpType.subtract
    mult = mybir.AluOpType.mult
    X = mybir.AxisListType.X

    nc = bass.Bass("TRN2", debug=False)

    ins = {}
    for names in BRANCHES:
        for n in names:
            ins[n] = nc.dram_tensor(n, [P, FREE], f32, kind="ExternalInput").ap()
    ins["yoh"] = nc.dram_tensor("yoh", [P, 2 * YF], f32, kind="ExternalInput").ap()
    ins["sz"] = nc.dram_tensor("sz", [P, SF], f32, kind="ExternalInput").ap()
    out_d = nc.dram_tensor("out", [P, 4 * NCOL], f32, kind="ExternalOutput").ap()

    with tile.TileContext(nc) as tc, ExitStack() as ctx:
        io = ctx.enter_context(tc.tile_pool(name="io", bufs=3))
        st = ctx.enter_context(tc.tile_pool(name="st", bufs=1))

        out_sb = st.tile([P, 4 * NCOL], f32, tag="out")

        for b, (mn, ln_, en, pn) in enumerate(BRANCHES):
            S_t = st.tile([P, NCOL], f32, tag=f"S{b}")
            PS_t = st.tile([P, NCOL], f32, tag=f"PS{b}")
            RD_t = st.tile([P, NCOL], f32, tag=f"RD{b}")

            for c in range(NCH):
                sl = bass.ts(c, CHUNK)
                gsl = bass.ts(c, G)

                m_t = io.tile([P, CHUNK], f32, tag="m")
                nc.sync.dma_start(m_t[:], ins[mn][:, sl])
                l_t = io.tile([P, CHUNK], f32, tag="l")
                nc.sync.dma_start(l_t[:], ins[ln_][:, sl])
                e_t = io.tile([P, CHUNK], f32, tag="e")
                nc.sync.dma_start(e_t[:], ins[en][:, sl])
                p_t = io.tile([P, CHUNK], f32, tag="p")
                nc.sync.dma_start(p_t[:], ins[pn][:, sl])

                # ACT: std = exp(0.5*log_std) in-place, then pe = exp(prior)
                nc.scalar.activation(l_t[:], l_t[:], Exp, scale=0.5)
                pe_t = io.tile([P, CHUNK], f32, tag="pe")
                nc.scalar.activation(pe_t[:], p_t[:], Exp)
                # DVE observes ACT here (covers std + pe ticks)
                nc.vector.tensor_reduce(
                    PS_t[:, gsl], pe_t[:].rearrange("p (g d) -> p g d", d=D), X, add
                )
                # se = std * eps        (into eps tile; waits only DMA-e)
                nc.vector.tensor_tensor(e_t[:], l_t[:], e_t[:], mult)
                # enc = se + mean       (into mean tile; waits only DMA-m)
                nc.vector.tensor_tensor(m_t[:], e_t[:], m_t[:], add)
                # d = enc - prior       (into prior tile; waits only DMA-p)
                nc.vector.tensor_tensor(p_t[:], m_t[:], p_t[:], sub)
                # e = exp(enc)          (ACT, into se tile; waits DVE)
                nc.scalar.activation(e_t[:], m_t[:], Exp)
                # DVE observes ACT e-tick
                nc.vector.tensor_reduce(
                    S_t[:, gsl], e_t[:].rearrange("p (g d) -> p g d", d=D), X, add
                )
                # ed = e * d            (into enc tile; all ticks observed)
                nc.vector.tensor_tensor(m_t[:], e_t[:], p_t[:], mult)
                nc.vector.tensor_reduce(
                    RD_t[:, gsl], m_t[:].rearrange("p (g d) -> p g d", d=D), X, add
                )

            # tail: kl_row = RD/S - ln S + ln PS
            rs_t = st.tile([P, NCOL], f32, tag=f"rs{b}")
            nc.vector.reciprocal(rs_t[:], S_t[:])
            term_t = st.tile([P, NCOL], f32, tag=f"term{b}")
            nc.vector.tensor_tensor(term_t[:], RD_t[:], rs_t[:], mult)
            lnS_t = st.tile([P, NCOL], f32, tag=f"lnS{b}")
            nc.scalar.activation(lnS_t[:], S_t[:], Ln)
            lnPS_t = st.tile([P, NCOL], f32, tag=f"lnPS{b}")
            nc.scalar.activation(lnPS_t[:], PS_t[:], Ln)
            tmp_t = st.tile([P, NCOL], f32, tag=f"tmp{b}")
            nc.vector.tensor_tensor(tmp_t[:], term_t[:], lnS_t[:], sub)
            nc.vector.tensor_tensor(
                out_sb[:, bass.ts(b, NCOL)], tmp_t[:], lnPS_t[:], add
            )

        # --- cross entropy on y_zt: per-row lse - picked ---
        yoh_t = st.tile([P, 2 * YF], f32, tag="yoh")
        nc.sync.dma_start(yoh_t[:], ins["yoh"][:])
        y_ap = yoh_t[:, 0:YF]
        oh_ap = yoh_t[:, YF:2 * YF]
        ey_t = st.tile([P, YF], f32, tag="ey")
        nc.scalar.activation(ey_t[:], y_ap, Exp)
        sy_t = st.tile([P, NCOL], f32, tag="sy")
        nc.vector.tensor_reduce(
            sy_t[:], ey_t[:].rearrange("p (g c) -> p g c", c=C), X, add
        )
        lse_t = st.tile([P, NCOL], f32, tag="lse")
        nc.scalar.activation(lse_t[:], sy_t[:], Ln)
        ym_t = st.tile([P, YF], f32, tag="ym")
        nc.vector.tensor_tensor(ym_t[:], y_ap, oh_ap, mult)
        pick_t = st.tile([P, NCOL], f32, tag="pick")
        nc.vector.tensor_reduce(
            pick_t[:], ym_t[:].rearrange("p (g c) -> p g c", c=C), X, add
        )
        nc.vector.tensor_tensor(
            out_sb[:, bass.ts(2, NCOL)], lse_t[:], pick_t[:], sub
        )

        # --- entropy of softmax(s_zt): per-row lse - (sum e*x)/s ---
        sz_t = st.tile([P, SF], f32, tag="sz")
        nc.sync.dma_start(sz_t[:], ins["sz"][:])
        esz_t = st.tile([P, SF], f32, tag="esz")
        nc.scalar.activation(esz_t[:], sz_t[:], Exp)
        ssum_t = st.tile([P, NCOL], f32, tag="ssum")
        nc.vector.tensor_reduce(
            ssum_t[:], esz_t[:].rearrange("p (g c) -> p g c", c=S), X, add
        )
        exs_t = st.tile([P, SF], f32, tag="exs")
        nc.vector.tensor_tensor(exs_t[:], esz_t[:], sz_t[:], mult)
        dsum_t = st.tile([P, NCOL], f32, tag="dsum")
        nc.vector.tensor_reduce(
            dsum_t[:], exs_t[:].rearrange("p (g c) -> p g c", c=S), X, add
        )
        rss_t = st.tile([P, NCOL], f32, tag="rss")
        nc.vector.reciprocal(rss_t[:], ssum_t[:])
        t2_t = st.tile([P, NCOL], f32, tag="t2")
        nc.vector.tensor_tensor(t2_t[:], dsum_t[:], rss_t[:], mult)
        lss_t = st.tile([P, NCOL], f32, tag="lss")
        nc.scalar.activation(lss_t[:], ssum_t[:], Ln)
        nc.vector.tensor_tensor(
            out_sb[:, bass.ts(3, NCOL)], lss_t[:], t2_t[:], sub
        )

        nc.sync.dma_start(out_d[:], out_sb[:])

    return nc


def _split_multi_waits(nc):
    """walrus's codegen allows a single embedded sync-wait per compute
    instruction; Tile sometimes emits two (e.g. ACT + DMA deps on one TT).
    Hoist all-but-one wait into standalone EventSemaphore instructions
    placed immediately before, on the same engine. Applied at BIR-JSON
    serialization time so CoreSim (which handles multi-wait fine) is
    untouched."""
    import json

    orig = nc.to_json_bytes

    def patched():
        bj = json.loads(orig())
        for fn in bj["functions"]:
            for blk in fn["blocks"]:
                new = []
                for inst in blk["instructions"]:
                    si = inst.get("sync_info") or {}
                    waits = si.get("on_wait") or []
                    if len(waits) > 1 and inst.get("opcode") != "EventSemaphore":
                        for i, w in enumerate(waits[:-1]):
                            new.append({
                                "debug": inst.get("debug"),
                                "engine": inst["engine"],
                                "ins": [],
                                "name": f"{inst['name']}-sw{i}",
                                "opcode": "EventSemaphore",
                                "outs": [],
                                "sync_info": {"on_update": [], "on_wait": [w]},
                            })
                        si["on_wait"] = [waits[-1]]
                    new.append(inst)
                blk["instructions"] = new
        return json.dumps(bj).encode()

    nc.to_json_bytes = patched
    return nc


def get_nc():
    global _CACHED_NC
    if _CACHED_NC is None:
        _CACHED_NC = _split_multi_waits(_build_nc())
    return _CACHED_NC


def make_in_maps(inputs):
    """Shard the full inputs into per-core in_maps for run_bass_kernel_spmd."""
    f32 = np.float32
    arr = {k: np.asarray(v) for k, v in inputs.items()}
    target = np.asarray(arr["target"]).astype(np.int64).reshape(B)
    onehot = np.zeros((B, C), dtype=f32)
    onehot[np.arange(B), target] = 1.0

    big = {
        "mt": arr["mean_t"], "lt": arr["log_std_t"],
        "et": arr["eps_t"], "ept": arr["eps_prior_t"],
        "ms": arr["mean_s"], "lsd": arr["log_std_s"],
        "ess": arr["eps_s"], "eps2": arr["eps_prior_s"],
    }
    in_maps = []
    for cidx in range(NCORES):
        sl = slice(cidx * RPC, (cidx + 1) * RPC)
        m = {
            k: np.ascontiguousarray(v[sl], dtype=f32).reshape(P, FREE)
            for k, v in big.items()
        }
        yoh = np.empty((P, 2 * YF), dtype=f32)
        yoh[:, :YF] = np.ascontiguousarray(arr["y_zt"][sl], dtype=f32).reshape(P, YF)
        yoh[:, YF:] = np.ascontiguousarray(onehot[sl]).reshape(P, YF)
        m["yoh"] = yoh
        m["sz"] = np.ascontiguousarray(arr["s_zt"][sl], dtype=f32).reshape(P, SF)
        in_maps.append(m)
    return in_maps


def combine(outs, current_step):
    """Host-side unshard: f64 reduce of per-row partials -> final f32 scalar."""
    tot = np.zeros(4, dtype=np.float64)
    for o in outs:
        o = o.reshape(P, 4, NCOL)
        tot += o.sum(axis=(0, 2), dtype=np.float64)
    L_zt, L_zs, L_t, Loss_e = tot / B
    frac = float(current_step) / STEP_SIZE
    lam_e = LAMBDA_E * GAMMA_E ** frac
    lam_od = LAMBDA_OD * GAMMA_OD ** frac
    val = L_t + lam_e * Loss_e + lam_od * (L_zt + L_zs)
    return np.array(val, dtype=np.float32)


def _install_ntff_hook():
    """Best-effort: register the axon NTFF profiling hook that the agent
    image's antenv package is missing, so trace=True yields exec_time_ns."""
    try:
        import sys, types
        import antenv
        if "antenv.axon_hooks" in sys.modules:
            return True
        sys.path.insert(0, "/root/.axon_site/trn_agent_boot")
        import trn_boot
        mod = types.ModuleType("antenv.axon_hooks")
        _h = {}
        mod.set_axon_ntff_profile_hook = lambda h: _h.__setitem__("h", h)
        mod.get_axon_ntff_profile_hook = lambda: _h.get("h")
        sys.modules["antenv.axon_hooks"] = mod
        antenv.axon_hooks = mod
        mod.set_axon_ntff_profile_hook(
            trn_boot._ntff_profile_via_ctypes("/opt/axon/libaxon_pjrt.so")
        )
        import concourse.bass_utils as bu
        bu.upload_artifacts = lambda tmpdir: str(tmpdir)
        return True
    except Exception:
        return False


def kernel(**inputs):
    global LAST_EXEC_NS
    from concourse.bass_utils import run_bass_kernel_spmd

    trace = os.environ.get("BASS_KERNEL_TRACE", "0") == "1"
    if trace:
        trace = _install_ntff_hook()

    nc = get_nc()
    in_maps = make_in_maps(inputs)
    res = run_bass_kernel_spmd(
        nc, in_maps, list(range(NCORES)), trace=trace
    )
    LAST_EXEC_NS = res.exec_time_ns
    outs = [r["out"] for r in res.results]
    cs = inputs.get("current_step", 500)
    return combine(outs, int(np.asarray(cs)))
